# revision 12
# baseline (speedup 1.0000x reference)
"""Trainium2 Bass kernel for nn_DifferentialMaxtree (N = 4M tree nodes).

Pipeline (8-way data-parallel over tree nodes, one shard per NeuronCore):
  A) linear/sigmoid head over the 15 raw attributes -> contrib = diff * score
     (the memory-dominant stage: streams the 251MB attribute matrix)
  B) tree filter out[i] = sum of contrib over i's ancestor chain, computed
     WITHOUT pointer chasing via a DFS-interval identity: relabel nodes in
     DFS preorder (host index prep); the subtree of node t is the contiguous
     interval [t, end_t], so
         out[i] = P1[i] - P2[i],
         P1 = prefix-sum(contrib),
         P2[i] = R[i - depth(i) - 1],
         R = prefix-sum of contrib permuted into end-sorted (postorder) order.
     The prefix sums (the O(N) value computation) run on device as streaming
     scans with cross-partition offsets folded in via a triangular matmul.

TRN2's DMA engines only support block-granular indirect addressing (one
latched offset per contiguous descriptor run), so the two data-dependent
permutations (contrib -> postorder and the R sampling at i-depth(i)-1) are
applied on the host between the two device launches, as part of the
shard/unshard glue; they are pure index-space reshuffles with no arithmetic.
"""

import math
import numpy as np

N = 4194304
H = W = 2048
NCORES = 8
P = 128
S = N // NCORES  # 524288 nodes per core
F = S // P  # 4096 free elems per partition
EPS = 1e-10

KT = 512  # stage-A rows per partition per tile
NT = F // KT  # stage-A tiles



# ---- inlined compat: this walrus build rejects >1 semaphore wait per
# instruction ("Too many sync wait commands"); split extras onto nops ----
def _split_excess_waits(nc, max_waits=1):
    from concourse import mybir

    counter = 0
    for f in nc.m.functions:
        for bb in f.blocks:
            il = bb.instructions
            i = 0
            while i < len(il):
                inst = il[i]
                si = inst.sync_info
                if si is not None and len(si.on_wait) > max_waits:
                    waits = list(si.on_wait)
                    rest, keep = waits[:-max_waits], waits[-max_waits:]
                    pre = []
                    for j in range(0, len(rest), max_waits):
                        nop = mybir.InstNoOp(
                            name=f"I-waitsplit-{counter}", ins=[], outs=[]
                        )
                        nop.engine = inst.engine
                        nop.sync_info = mybir.SyncInfo(
                            on_wait=list(rest[j : j + max_waits]), on_update=[]
                        )
                        counter += 1
                        pre.append(nop)
                    inst.sync_info = mybir.SyncInfo(
                        on_wait=keep, on_update=list(si.on_update)
                    )
                    for k, p_ in enumerate(pre):
                        il.insert(i + k, p_)
                    i += len(pre)
                i += 1
    return counter


def _host_prep(parent):
    """DFS relabeling, interval ends, postorder rank and sample positions."""
    import scipy.sparse as sp
    from scipy.sparse.csgraph import depth_first_order

    parent = np.asarray(parent).astype(np.int64)
    idx = np.arange(1, N, dtype=np.int64)
    g = sp.csr_matrix((np.ones(N - 1, np.int8), (parent[1:], idx)), shape=(N, N))
    order = np.asarray(
        depth_first_order(g, 0, directed=True, return_predecessors=False),
        dtype=np.int64,
    )
    assert order.shape[0] == N, f"tree not rooted/connected: {order.shape}"

    # depth (number of proper ancestors) via pointer doubling
    SENT = N
    p = np.concatenate([parent, [SENT]])
    p[0] = SENT
    chains = []
    pk = p.copy()
    while not np.all(pk[:N] == SENT):
        chains.append(pk.copy())
        pk = pk[pk]
        pk[SENT] = SENT
    depth = np.zeros(N + 1, np.int64)
    cur = np.arange(N + 1)
    for k in range(len(chains) - 1, -1, -1):
        anc = chains[k][cur]
        mask = anc != SENT
        depth[mask] += 1 << k
        cur = np.where(mask, anc, cur)
    d_old = depth[:N]

    # subtree sizes: accumulate child -> parent, deepest level first
    size = np.ones(N, np.int64)
    dorder = np.argsort(d_old, kind="stable")
    maxd = int(d_old.max())
    dstarts = np.searchsorted(d_old[dorder], np.arange(maxd + 2))
    for lev in range(maxd, 0, -1):
        nodes = dorder[dstarts[lev] : dstarts[lev + 1]]
        np.add.at(size, parent[nodes], size[nodes])

    end_new = np.arange(N, dtype=np.int64) + size[order] - 1
    d_new = d_old[order]
    sigma = np.argsort(end_new, kind="stable")  # rank r -> source node t
    q = np.arange(N, dtype=np.int64) - d_new - 1  # P2 sample position (rho-1)
    return {"order": order, "sigma": sigma, "q": q}


def _build_stage_a(w, b, repeat=1):
    """Program A: attribute head -> contrib, per core shard."""
    from concourse import bass, mybir
    import concourse.tile as tile

    DT = mybir.dt.float32
    AF = mybir.ActivationFunctionType
    OP = mybir.AluOpType

    w = [float(x) for x in w]
    b = float(b)

    nc = bass.Bass()
    attr_d = nc.declare_dram_parameter("attr", [S * 15], DT, isOutput=False)
    diff_d = nc.declare_dram_parameter("diff", [S], DT, isOutput=False)
    con_d = nc.declare_dram_parameter("contrib", [S], DT, isOutput=True)

    # ACT biases must exist as const APs
    for _cv in (EPS, math.pi / 2):
        _ct = nc.alloc_sbuf_tensor(f"const-f32-{_cv}", [P, 1], DT)
        nc.gpsimd.memset(_ct.ap(), _cv)
        nc.const_aps.aps[(DT, _cv)] = _ct.ap()
    nc.all_engine_barrier()

    with tile.TileContext(nc) as tc:
        with tc.tile_pool(name="sbuf", bufs=2) as pool:
            for _rep in range(repeat):
                attr_v = attr_d[:].rearrange("(p f c) -> p (f c)", p=P, c=15)
                diff_v = diff_d[:].rearrange("(p f) -> p f", p=P)
                con_v = con_d[:].rearrange("(p f) -> p f", p=P)
                for t in range(NT):
                    K = KT
                    at = pool.tile([P, K * 15], DT, tag="at")
                    nc.sync.dma_start(
                        out=at[:], in_=attr_v[:, t * K * 15 : (t + 1) * K * 15]
                    )
                    dt_ = pool.tile([P, K], DT, tag="dt")
                    nc.sync.dma_start(
                        out=dt_[:], in_=diff_v[:, t * K : (t + 1) * K]
                    )
                    ba = at[:]
                    pd = list(ba.ap[0])

                    def col(j):  # strided [P, K] view of raw column j
                        return bass.AP(
                            ba.tensor, ba.offset + j, [pd, [15, K]]
                        )

                    col9 = bass.AP(  # columns 6..14 as [P, 9, K]
                        ba.tensor, ba.offset + 6, [pd, [1, 9], [15, K]]
                    )
                    feat = pool.tile([P, 12 * K], DT, tag="feat")
                    feat3 = feat[:].rearrange("p (c k) -> p c k", k=K)
                    nc.scalar.activation(
                        feat3[:, 0:9, :], col9, AF.Ln, bias=EPS
                    )
                    nc.scalar.activation(
                        feat[:, 9 * K : 10 * K], col(5), AF.Sin,
                        bias=math.pi / 2,
                    )
                    nc.scalar.activation(
                        feat[:, 10 * K : 11 * K], col(5), AF.Sin
                    )
                    sq7 = pool.tile([P, K], DT, tag="sq7")
                    sq6 = pool.tile([P, K], DT, tag="sq6")
                    nc.scalar.activation(sq7[:], col(7), AF.Sqrt)
                    nc.scalar.activation(sq6[:], col(6), AF.Sqrt)
                    nc.vector.tensor_scalar_add(sq6[:], sq6[:], EPS)
                    nc.vector.reciprocal(sq6[:], sq6[:])
                    nc.vector.tensor_tensor(
                        out=feat[:, 11 * K : 12 * K], in0=sq7[:], in1=sq6[:],
                        op=OP.mult,
                    )
                    y = pool.tile([P, K], DT, tag="y")
                    nc.scalar.activation(
                        y[:], col(0), AF.Copy, bias=b, scale=w[0]
                    )
                    y1 = pool.tile([P, K], DT, tag="y1")
                    nc.vector.tensor_scalar(
                        out=y1[:], in0=col(1), scalar1=w[1], scalar2=None,
                        op0=OP.mult,
                    )
                    # two independent accumulator chains (y even, y1 odd)
                    terms = [("c", j) for j in range(2, 5)] + [
                        ("f", j) for j in range(12)
                    ]
                    for n, (kind, j) in enumerate(terms):
                        src = (
                            col(j)
                            if kind == "c"
                            else feat[:, j * K : (j + 1) * K]
                        )
                        wt = w[j] if kind == "c" else w[5 + j]
                        acc = y if n % 2 == 0 else y1
                        nc.vector.scalar_tensor_tensor(
                            out=acc[:], in0=src, scalar=wt, in1=acc[:],
                            op0=OP.mult, op1=OP.add,
                        )
                    nc.vector.tensor_tensor(
                        out=y[:], in0=y[:], in1=y1[:], op=OP.add
                    )
                    sc = pool.tile([P, K], DT, tag="sc")
                    nc.scalar.activation(sc[:], y[:], AF.Sigmoid)
                    ct = pool.tile([P, K], DT, tag="ct")
                    nc.vector.tensor_tensor(
                        out=ct[:], in0=sc[:], in1=dt_[:], op=OP.mult
                    )
                    nc.sync.dma_start(
                        out=con_v[:, t * K : (t + 1) * K], in_=ct[:]
                    )

    _split_excess_waits(nc)
    return nc


def _build_scans(repeat=1):
    """Program B: prefix scans of contrib (P1) and postorder contrib (R),
    with cross-partition offsets folded in via a triangular matmul."""
    from concourse import bass, mybir
    import concourse.tile as tile

    DT = mybir.dt.float32
    OP = mybir.AluOpType

    nc = bass.Bass()
    con_d = nc.declare_dram_parameter("contrib", [S], DT, isOutput=False)
    cs_d = nc.declare_dram_parameter("cs", [S], DT, isOutput=False)
    triu_d = nc.declare_dram_parameter("triu", [P, P], DT, isOutput=False)
    p1_d = nc.declare_dram_parameter("p1a", [S], DT, isOutput=True)
    ra_d = nc.declare_dram_parameter("ra", [S], DT, isOutput=True)

    with tile.TileContext(nc) as tc:
        with (
            tc.tile_pool(name="sbuf", bufs=2) as pool,
            tc.tile_pool(name="perm", bufs=1) as perm,
            tc.tile_pool(name="psum", bufs=2, space="PSUM") as psum,
        ):
            for _rep in range(repeat):
                con = perm.tile([P, F], DT)
                nc.sync.dma_start(
                    out=con[:], in_=con_d[:].rearrange("(p f) -> p f", p=P)
                )
                cs = perm.tile([P, F], DT)
                nc.sync.dma_start(
                    out=cs[:], in_=cs_d[:].rearrange("(p f) -> p f", p=P)
                )
                zt = perm.tile([P, F], DT)
                nc.vector.memset(zt[:], 0.0)

                # two-level scan: 32 segment scans of 128 + offset fold
                # (sequential fp32 drift over 4096 elems is too lossy)
                NSEG = 32
                LS = F // NSEG

                def scan2(dst, src):
                    segt = pool.tile([P, NSEG], DT, tag="segt")
                    for s in range(NSEG):
                        sl = slice(s * LS, (s + 1) * LS)
                        nc.vector.tensor_tensor_scan(
                            out=dst[:, sl], data0=src[:, sl],
                            data1=zt[:, sl], initial=0.0,
                            op0=OP.add, op1=OP.add,
                        )
                    dv = dst[:]
                    tails = bass.AP(
                        dv.tensor, dv.offset + LS - 1,
                        [list(dv.ap[0]), [LS, NSEG]],
                    )
                    nc.vector.tensor_copy(out=segt[:], in_=tails)
                    sego = pool.tile([P, NSEG], DT, tag="sego")
                    nc.vector.tensor_tensor_scan(
                        out=sego[:], data0=segt[:], data1=zt[:, 0:NSEG],
                        initial=0.0, op0=OP.add, op1=OP.add,
                    )
                    for s in range(1, NSEG):
                        sl = slice(s * LS, (s + 1) * LS)
                        nc.vector.tensor_scalar(
                            out=dst[:, sl], in0=dst[:, sl],
                            scalar1=sego[:, s - 1 : s], scalar2=None,
                            op0=OP.add,
                        )

                p1 = perm.tile([P, F], DT)
                scan2(p1, con)
                rsc = perm.tile([P, F], DT)
                scan2(rsc, cs)
                triu = perm.tile([P, P], DT)
                nc.sync.dma_start(out=triu[:], in_=triu_d[:])
                tots = pool.tile([P, 2], DT)
                nc.vector.tensor_copy(out=tots[:, 0:1], in_=p1[:, F - 1 : F])
                nc.vector.tensor_copy(out=tots[:, 1:2], in_=rsc[:, F - 1 : F])
                po = psum.tile([P, 2], DT)
                nc.tensor.matmul(
                    out=po[:], lhsT=triu[:], rhs=tots[:], start=True, stop=True
                )
                pos = pool.tile([P, 2], DT)
                nc.vector.tensor_copy(out=pos[:], in_=po[:])
                nc.vector.tensor_scalar(
                    out=p1[:], in0=p1[:], scalar1=pos[:, 0:1], scalar2=None,
                    op0=OP.add,
                )
                nc.vector.tensor_scalar(
                    out=rsc[:], in0=rsc[:], scalar1=pos[:, 1:2], scalar2=None,
                    op0=OP.add,
                )
                nc.sync.dma_start(
                    out=p1_d[:].rearrange("(p f) -> p f", p=P), in_=p1[:]
                )
                nc.sync.dma_start(
                    out=ra_d[:].rearrange("(p f) -> p f", p=P), in_=rsc[:]
                )

    _split_excess_waits(nc)
    return nc


def _prepare_inputs(maxtree_parent, maxtree_diff, attributes):
    diff = np.asarray(maxtree_diff, dtype=np.float32)
    attrs = np.ascontiguousarray(np.asarray(attributes, dtype=np.float32))
    prep = _host_prep(maxtree_parent)
    order = prep["order"]
    attr_p = attrs[order]
    diff_p = diff[order]
    in_maps_a = [
        {
            "attr": attr_p[c * S : (c + 1) * S].reshape(-1),
            "diff": diff_p[c * S : (c + 1) * S],
        }
        for c in range(NCORES)
    ]
    return in_maps_a, prep


def _run_device(in_maps_a, prep, w, b, repeat=1, progs=None):
    """Run both device programs; host applies the index permutations between
    them.  Returns (out_new, progs) where progs can be reused for re-runs."""
    from concourse.bass_utils import run_bass_kernel_spmd

    cores = list(range(NCORES))
    if progs is None:
        progs = (_build_stage_a(w, b, repeat), _build_scans(repeat))
    nc_a, nc_b = progs

    res_a = run_bass_kernel_spmd(nc_a, in_maps_a, cores)
    contrib = np.concatenate(
        [res_a.results[c]["contrib"] for c in range(NCORES)]
    )

    cs = contrib[prep["sigma"]]  # postorder permutation (host, index-only)
    triu = np.triu(np.ones((P, P), np.float32), 1)
    in_maps_b = [
        {
            "contrib": contrib[c * S : (c + 1) * S],
            "cs": cs[c * S : (c + 1) * S],
            "triu": triu,
        }
        for c in range(NCORES)
    ]
    res_b = run_bass_kernel_spmd(nc_b, in_maps_b, cores)

    # host: fold core-level offsets, sample R, combine (index glue + O(N) adds)
    p1a = np.concatenate([res_b.results[c]["p1a"] for c in range(NCORES)])
    ra = np.concatenate([res_b.results[c]["ra"] for c in range(NCORES)])
    t1 = p1a[S - 1 :: S].astype(np.float32)
    t2 = ra[S - 1 :: S].astype(np.float32)
    o1 = np.repeat(
        np.concatenate([[0], np.cumsum(t1[:-1])]).astype(np.float32), S
    )
    o2 = np.repeat(
        np.concatenate([[0], np.cumsum(t2[:-1])]).astype(np.float32), S
    )
    rg = (ra + o2).astype(np.float32)
    q = prep["q"]
    p2 = np.where(q >= 0, rg[np.maximum(q, 0)], np.float32(0.0))
    out_new = ((p1a + o1) - p2).astype(np.float32)
    return out_new, progs


def kernel(maxtree_parent, maxtree_diff, attributes, weight, bias):
    w = np.asarray(weight, dtype=np.float32)[:, 0]
    b = float(np.asarray(bias, dtype=np.float32)[0])
    in_maps_a, prep = _prepare_inputs(
        maxtree_parent, maxtree_diff, attributes
    )
    out_new, _ = _run_device(in_maps_a, prep, w, b)
    out = np.empty(N, np.float32)
    out[prep["order"]] = out_new
    return out.reshape(H, W)
